# revision 3
# baseline (speedup 1.0000x reference)
"""Trainium2 Bass kernel for nn_Decoder (2-layer LSTM decoder, B=512, T=256, H=1024).

Strategy: 8-way tensor parallelism over the 4H gate dimension.
  - Core r holds gate rows [g*H + r*128 : g*H + (r+1)*128] for g in {i,f,g,o}
    of every weight matrix (transposed into lhsT layout, resident in SBUF).
  - Activations are kept "hidden-major": hT [H, B] so the batch (512) is the
    matmul moving/free dimension (full-rate float32r matmuls).
  - Each step, each core computes its 512-row gate slice for the full batch,
    the cell update for its 128-row h-slice, then an 8-rank AllGather of the
    h-slice (through HBM) so every core has the full h for the next matmuls.
  - Emission is software-pipelined so each AllGather hides behind the other
    layer's recurrent matmul.
  - The decoder feedback x_t = fc_out(h1_{t-1}) is computed on-device.
  - The init linear (fc_init) and final [T,64,B] -> [B,T,64] transpose happen
    host-side.
"""

import sys
import time

import numpy as np

if "/opt/trn_rl_repo" not in sys.path:
    sys.path.insert(0, "/opt/trn_rl_repo")

B = 512
T_FULL = 256
H = 1024
OUT = 64
NCORES = 8
HC = H // NCORES  # 128 hidden rows per core
KH = H // 128  # 8 K-tiles over hidden dim
NM = 4  # 4 M-tiles (one per gate) in a core's 512-row gate slice

_CACHE = {}


def _build(T, n_chunks):
    import concourse.bacc as bacc
    import concourse.tile as tile
    import concourse.mybir as mybir

    F32 = mybir.dt.float32
    F32R = mybir.dt.float32r
    AF = mybir.ActivationFunctionType
    ALU = mybir.AluOpType
    RG = [list(range(NCORES))]

    nc = bacc.Bacc(
        "TRN2",
        target_bir_lowering=False,
        debug=False,
        enable_asserts=False,
        num_devices=NCORES,
    )

    def din(name, shape, dt=F32R):
        return nc.dram_tensor(name, shape, dt, kind="ExternalInput").ap()

    wih0_d = din("wih0", [OUT, 4 * HC])
    whh0_d = din("whh0", [H, 4 * HC])
    wih1_d = din("wih1", [H, 4 * HC])
    whh1_d = din("whh1", [H, 4 * HC])
    wout_d = din("wout", [H, OUT])
    bias0_d = din("bias0", [HC, NM], F32)
    bias1_d = din("bias1", [HC, NM], F32)
    bout_d = din("bout", [OUT, 1], F32)
    h0i_d = din("h0i", [H, B])
    h1i_d = din("h1i", [H, B])
    c0i_d = din("c0i", [HC, B], F32)
    c1i_d = din("c1i", [HC, B], F32)
    xi_d = din("xi", [OUT, B])

    outs_d = nc.dram_tensor("outsT", [T, OUT, B], F32R, kind="ExternalOutput").ap()
    state_out = {}
    if n_chunks > 1:
        state_out["h0o"] = nc.dram_tensor("h0o", [H, B], F32R, kind="ExternalOutput").ap()
        state_out["h1o"] = nc.dram_tensor("h1o", [H, B], F32R, kind="ExternalOutput").ap()
        state_out["c0o"] = nc.dram_tensor("c0o", [HC, B], F32, kind="ExternalOutput").ap()
        state_out["c1o"] = nc.dram_tensor("c1o", [HC, B], F32, kind="ExternalOutput").ap()
        state_out["xo"] = nc.dram_tensor("xo", [OUT, B], F32R, kind="ExternalOutput").ap()

    with tile.TileContext(nc) as tc:
        with (
            tc.tile_pool(name="wp", bufs=1) as wp,
            tc.tile_pool(name="hp", bufs=2) as hp,
            tc.tile_pool(name="gp", bufs=2) as gp,
            tc.tile_pool(name="ps", bufs=7, space="PSUM") as ps,
            tc.tile_pool(name="dp", bufs=2, space="DRAM") as dp,
        ):
            # ---- weight / bias / init preload ----
            wih0 = wp.tile([OUT, NM, 128], F32R, name="wih0_s")
            nc.sync.dma_start(wih0[:], wih0_d.rearrange("k (m q) -> k m q", q=128))
            whh0 = wp.tile([128, KH, NM, 128], F32R, name="whh0_s")
            nc.sync.dma_start(
                whh0[:], whh0_d.rearrange("(k p) (m q) -> p k m q", p=128, q=128)
            )
            wih1 = wp.tile([128, KH, NM, 128], F32R, name="wih1_s")
            nc.sync.dma_start(
                wih1[:], wih1_d.rearrange("(k p) (m q) -> p k m q", p=128, q=128)
            )
            whh1 = wp.tile([128, KH, NM, 128], F32R, name="whh1_s")
            nc.sync.dma_start(
                whh1[:], whh1_d.rearrange("(k p) (m q) -> p k m q", p=128, q=128)
            )
            wout = wp.tile([128, KH, OUT], F32R, name="wout_s")
            nc.sync.dma_start(wout[:], wout_d.rearrange("(k p) o -> p k o", p=128))
            bias0 = wp.tile([HC, NM], F32, name="bias0_s")
            nc.sync.dma_start(bias0[:], bias0_d)
            bias1 = wp.tile([HC, NM], F32, name="bias1_s")
            nc.sync.dma_start(bias1[:], bias1_d)
            bout = wp.tile([OUT, 1], F32, name="bout_s")
            nc.sync.dma_start(bout[:], bout_d)

            h0_full = hp.tile([128, KH, B], F32R, name="h0f", tag="h0f")
            nc.sync.dma_start(h0_full[:], h0i_d.rearrange("(k p) n -> p k n", p=128))
            h1_full = hp.tile([128, KH, B], F32R, name="h1f", tag="h1f")
            nc.sync.dma_start(h1_full[:], h1i_d.rearrange("(k p) n -> p k n", p=128))
            c0 = gp.tile([HC, B], F32, name="c0", tag="c0")
            nc.sync.dma_start(c0[:], c0i_d)
            c1 = gp.tile([HC, B], F32, name="c1", tag="c1")
            nc.sync.dma_start(c1[:], c1i_d)
            x = gp.tile([OUT, B], F32R, name="x", tag="x")
            nc.sync.dma_start(x[:], xi_d)

            def mm_hh(psums, w_s, h_tile, start):
                # m-outer, k-inner: each gate's PSUM finishes early
                for m in range(NM):
                    for k in range(KH):
                        nc.tensor.matmul(
                            psums[m][:],
                            w_s[:, k, m, :],
                            h_tile[:, k, :],
                            start=(start and k == 0),
                            stop=False,
                        )

            def mm_ih1(psums, h_tile):
                # k-outer, m-inner: consume AllGather-DMA'd tiles as they land
                for k in range(KH):
                    for m in range(NM):
                        nc.tensor.matmul(
                            psums[m][:],
                            wih1[:, k, m, :],
                            h_tile[:, k, :],
                            start=False,
                            stop=(k == KH - 1),
                        )

            def mm_ih0(psums, x_tile):
                for m in range(NM):
                    nc.tensor.matmul(
                        psums[m][:],
                        wih0[:, m, :],
                        x_tile[:],
                        start=False,
                        stop=True,
                    )

            def cell(layer, psums, bias_s, c_prev):
                """LSTM cell elementwise math for this core's 128-row h slice."""
                gi = gp.tile([HC, B], F32, name=f"gi{layer}", tag=f"gi{layer}", bufs=1)
                nc.scalar.activation(gi[:], psums[0][:], AF.Sigmoid, bias=bias_s[:, 0:1])
                gf = gp.tile([HC, B], F32, name=f"gf{layer}", tag=f"gf{layer}", bufs=1)
                nc.scalar.activation(gf[:], psums[1][:], AF.Sigmoid, bias=bias_s[:, 1:2])
                gg = gp.tile([HC, B], F32, name=f"gg{layer}", tag=f"gg{layer}", bufs=1)
                nc.scalar.activation(gg[:], psums[2][:], AF.Tanh, bias=bias_s[:, 2:3])
                go = gp.tile([HC, B], F32, name=f"go{layer}", tag=f"go{layer}", bufs=1)
                nc.scalar.activation(go[:], psums[3][:], AF.Sigmoid, bias=bias_s[:, 3:4])
                t1 = gp.tile([HC, B], F32, name=f"t1_{layer}", tag=f"t1_{layer}", bufs=1)
                nc.vector.tensor_tensor(t1[:], gf[:], c_prev[:], ALU.mult)
                t2 = gp.tile([HC, B], F32, name=f"t2_{layer}", tag=f"t2_{layer}", bufs=1)
                nc.vector.tensor_tensor(t2[:], gi[:], gg[:], ALU.mult)
                c_new = gp.tile([HC, B], F32, name=f"c{layer}", tag=f"c{layer}")
                nc.vector.tensor_tensor(c_new[:], t1[:], t2[:], ALU.add)
                tc_t = gp.tile([HC, B], F32, name=f"tc{layer}", tag=f"tc{layer}", bufs=1)
                nc.scalar.activation(tc_t[:], c_new[:], AF.Tanh)
                h_r = gp.tile([HC, B], F32R, name=f"hr{layer}", tag=f"hr{layer}")
                nc.vector.tensor_tensor(h_r[:], go[:], tc_t[:], ALU.mult)
                return c_new, h_r

            def allgather(layer, h_r):
                agin = dp.tile([HC, B], F32R, name=f"agin{layer}", tag=f"agin{layer}")
                agout = dp.tile(
                    [H, B],
                    F32R,
                    name=f"agout{layer}",
                    tag=f"agout{layer}",
                    addr_space="Shared",
                )
                nc.sync.dma_start(agin[:], h_r[:])
                nc.gpsimd.collective_compute(
                    "AllGather",
                    ALU.bypass,
                    replica_groups=RG,
                    ins=[agin.opt()],
                    outs=[agout.opt()],
                )
                hf = hp.tile([128, KH, B], F32R, name=f"h{layer}f", tag=f"h{layer}f")
                for k in range(KH):
                    nc.sync.dma_start(hf[:, k, :], agout[k * 128 : (k + 1) * 128, :])
                return hf

            def gate_psums(layer):
                return [
                    ps.tile([HC, B], F32, name=f"ps{layer}_{m}", tag="gates", bufs=7)
                    for m in range(NM)
                ]

            # ---- prologue: layer 0 of step 0 ----
            g0 = gate_psums(0)
            mm_hh(g0, whh0, h0_full, start=True)
            mm_ih0(g0, x)
            c0, hr0 = cell(0, g0, bias0, c0)
            h0_full = allgather(0, hr0)

            # ---- main loop ----
            for t in range(T):
                g1 = gate_psums(1)
                mm_hh(g1, whh1, h1_full, start=True)
                mm_ih1(g1, h0_full)
                c1, hr1 = cell(1, g1, bias1, c1)
                h1_full = allgather(1, hr1)

                if t < T - 1:
                    g0 = gate_psums(0)
                    mm_hh(g0, whh0, h0_full, start=True)

                po = ps.tile([OUT, B], F32, name="po", tag="pout", bufs=1)
                for k in range(KH):
                    nc.tensor.matmul(
                        po[:],
                        wout[:, k, :],
                        h1_full[:, k, :],
                        start=(k == 0),
                        stop=(k == KH - 1),
                    )
                x = gp.tile([OUT, B], F32R, name="x", tag="x")
                nc.scalar.activation(x[:], po[:], AF.Identity, bias=bout[:])
                nc.sync.dma_start(outs_d[t], x[:])

                if t < T - 1:
                    mm_ih0(g0, x)
                    c0, hr0 = cell(0, g0, bias0, c0)
                    h0_full = allgather(0, hr0)

            # ---- state outputs for chunked execution ----
            if n_chunks > 1:
                nc.sync.dma_start(
                    state_out["h0o"].rearrange("(k p) n -> p k n", p=128), h0_full[:]
                )
                nc.sync.dma_start(
                    state_out["h1o"].rearrange("(k p) n -> p k n", p=128), h1_full[:]
                )
                nc.sync.dma_start(state_out["c0o"], c0[:])
                nc.sync.dma_start(state_out["c1o"], c1[:])
                nc.sync.dma_start(state_out["xo"], x[:])

    nc.compile()
    return nc


def _get_nc(T, n_chunks):
    key = (T, n_chunks)
    if key not in _CACHE:
        _CACHE[key] = _build(T, n_chunks)
    return _CACHE[key]


def _run(nc, in_maps):
    from concourse import bass_utils

    return bass_utils.run_bass_kernel_spmd(
        nc, in_maps, core_ids=list(range(NCORES))
    ).results


def kernel(
    z_style,
    z_skill,
    fc_init_w,
    fc_init_b,
    W_ih0,
    W_hh0,
    b_ih0,
    b_hh0,
    W_ih1,
    W_hh1,
    b_ih1,
    b_hh1,
    fc_out_w,
    fc_out_b,
    T=T_FULL,
    n_chunks=1,
):
    f32 = np.float32
    z = np.concatenate([np.asarray(z_style, f32), np.asarray(z_skill, f32)], axis=1)
    init_flat = z @ np.asarray(fc_init_w, f32).T + np.asarray(fc_init_b, f32)
    h_all = init_flat.reshape(2, -1, H)  # faithful torch-style row-major view
    h0T = np.ascontiguousarray(h_all[0].T)  # [H, B]
    h1T = np.ascontiguousarray(h_all[1].T)

    bias0 = np.asarray(b_ih0, f32) + np.asarray(b_hh0, f32)
    bias1 = np.asarray(b_ih1, f32) + np.asarray(b_hh1, f32)
    wout = np.ascontiguousarray(np.asarray(fc_out_w, f32).T)  # [H, OUT]
    bout = np.asarray(fc_out_b, f32).reshape(OUT, 1)

    base_maps = []
    for r in range(NCORES):
        rows = np.concatenate(
            [np.arange(g * H + r * HC, g * H + (r + 1) * HC) for g in range(4)]
        )
        m = {
            "wih0": np.ascontiguousarray(np.asarray(W_ih0, f32)[rows].T),
            "whh0": np.ascontiguousarray(np.asarray(W_hh0, f32)[rows].T),
            "wih1": np.ascontiguousarray(np.asarray(W_ih1, f32)[rows].T),
            "whh1": np.ascontiguousarray(np.asarray(W_hh1, f32)[rows].T),
            "wout": wout,
            "bias0": np.ascontiguousarray(bias0[rows].reshape(NM, HC).T),
            "bias1": np.ascontiguousarray(bias1[rows].reshape(NM, HC).T),
            "bout": bout,
        }
        base_maps.append(m)

    state = {
        "h0i": h0T,
        "h1i": h1T,
        "c0i": np.zeros((HC, B), f32),
        "c1i": np.zeros((HC, B), f32),
        "xi": np.zeros((OUT, B), f32),
    }

    assert T % n_chunks == 0
    t_chunk = T // n_chunks
    nc = _get_nc(t_chunk, n_chunks)

    # per-core c state (each core carries its own 128-row slice)
    c0_state = [state["c0i"]] * NCORES
    c1_state = [state["c1i"]] * NCORES

    outs = []
    for _ in range(n_chunks):
        in_maps = []
        for r in range(NCORES):
            m = dict(base_maps[r])
            m.update(
                {
                    "h0i": state["h0i"],
                    "h1i": state["h1i"],
                    "xi": state["xi"],
                    "c0i": c0_state[r],
                    "c1i": c1_state[r],
                }
            )
            in_maps.append(m)
        res = _run(nc, in_maps)
        outs.append(res[0]["outsT"])
        if n_chunks > 1:
            state = {
                "h0i": res[0]["h0o"],
                "h1i": res[0]["h1o"],
                "xi": res[0]["xo"],
            }
            c0_state = [res[r]["c0o"] for r in range(NCORES)]
            c1_state = [res[r]["c1o"] for r in range(NCORES)]
    outsT = np.concatenate(outs, axis=0)  # [T, OUT, B]
    return np.ascontiguousarray(np.transpose(outsT, (2, 0, 1)))  # [B, T, OUT]


# revision 7
# speedup vs baseline: 1.5440x; 1.5440x over previous
"""Trainium2 Bass kernel for nn_Decoder (2-layer LSTM decoder, B=512, T=256, H=1024).

Strategy: 8-way tensor parallelism over the 4H gate dimension.
  - Core r holds gate rows [g*H + r*128 : g*H + (r+1)*128] for g in (i,f,o,g)
    of every weight matrix (transposed into lhsT layout, resident in SBUF).
  - Activations stay "hidden-major": hT [H, B], so the batch (512) is the
    matmul moving/free dimension (full-rate float32r matmuls, N=512).
  - Each step a core computes its 512-row gate slice for the full batch into
    one 4-bank PSUM tile, applies sigmoid/tanh (batched, gate order i,f,o,g),
    updates its 128-row c/h slice, then AllGathers the h slices (8 ranks,
    through HBM) so every core has the full h for the next step's matmuls.
  - Emission is software-pipelined: each AllGather hides behind the other
    layer's recurrent matmul (which only needs locally-available state).
  - The decoder feedback x_t = fc_out(h1_{t-1}) is computed on-device.
  - The init linear (fc_init) and the final [T,64,B] -> [B,T,64] transpose
    happen host-side.
"""

import sys

import numpy as np

if "/opt/trn_rl_repo" not in sys.path:
    sys.path.insert(0, "/opt/trn_rl_repo")

B = 512
T_FULL = 256
H = 1024
OUT = 64
NCORES = 8
HC = H // NCORES  # 128 hidden rows per core
KH = H // 128  # 8 K-tiles over the hidden dim
NM = 4  # 4 M-tiles (one per gate) in a core's 512-row gate slice
GW = NM * B  # gates tile free width (2048)

_CACHE = {}


def _build(T, n_chunks, zero_bias):
    import concourse.bacc as bacc
    import concourse.tile as tile
    import concourse.mybir as mybir

    F32 = mybir.dt.float32
    F32R = mybir.dt.float32r
    AF = mybir.ActivationFunctionType
    ALU = mybir.AluOpType
    RG = [list(range(NCORES))]

    nc = bacc.Bacc(
        "TRN2",
        target_bir_lowering=False,
        debug=False,
        enable_asserts=False,
        num_devices=NCORES,
    )

    def din(name, shape, dt=F32R):
        return nc.dram_tensor(name, shape, dt, kind="ExternalInput").ap()

    wih0_d = din("wih0", [OUT, 4 * HC])
    whh0_d = din("whh0", [H, 4 * HC])
    wih1_d = din("wih1", [H, 4 * HC])
    whh1_d = din("whh1", [H, 4 * HC])
    wout_d = din("wout", [H, OUT])
    bias0_d = din("bias0", [HC, NM], F32)
    bias1_d = din("bias1", [HC, NM], F32)
    bout_d = din("bout", [OUT, 1], F32)
    h0i_d = din("h0i", [H, B])
    h1i_d = din("h1i", [H, B])
    c0i_d = din("c0i", [HC, B], F32)
    c1i_d = din("c1i", [HC, B], F32)
    xi_d = din("xi", [OUT, B])

    outs_d = nc.dram_tensor("outsT", [T, OUT, B], F32R, kind="ExternalOutput").ap()
    so = {}
    if n_chunks > 1:
        so["h0o"] = nc.dram_tensor("h0o", [H, B], F32R, kind="ExternalOutput").ap()
        so["h1o"] = nc.dram_tensor("h1o", [H, B], F32R, kind="ExternalOutput").ap()
        so["c0o"] = nc.dram_tensor("c0o", [HC, B], F32, kind="ExternalOutput").ap()
        so["c1o"] = nc.dram_tensor("c1o", [HC, B], F32, kind="ExternalOutput").ap()
        so["xo"] = nc.dram_tensor("xo", [OUT, B], F32R, kind="ExternalOutput").ap()

    with tile.TileContext(nc) as tc:
        with (
            tc.tile_pool(name="wp", bufs=1) as wp,
            tc.tile_pool(name="hp", bufs=2) as hp,
            tc.tile_pool(name="gp", bufs=1) as gp,
            tc.tile_pool(name="ps", bufs=2, space="PSUM") as ps,
            tc.tile_pool(name="dp", bufs=2, space="DRAM") as dp,
        ):
            # ---- weight / bias / initial-state preload ----
            wih0 = wp.tile([OUT, NM, 128], F32R, name="wih0_s")
            nc.sync.dma_start(wih0[:], wih0_d.rearrange("k (m q) -> k m q", q=128))
            whh0 = wp.tile([128, KH, NM, 128], F32R, name="whh0_s")
            nc.sync.dma_start(
                whh0[:], whh0_d.rearrange("(k p) (m q) -> p k m q", p=128, q=128)
            )
            wih1 = wp.tile([128, KH, NM, 128], F32R, name="wih1_s")
            nc.sync.dma_start(
                wih1[:], wih1_d.rearrange("(k p) (m q) -> p k m q", p=128, q=128)
            )
            whh1 = wp.tile([128, KH, NM, 128], F32R, name="whh1_s")
            nc.sync.dma_start(
                whh1[:], whh1_d.rearrange("(k p) (m q) -> p k m q", p=128, q=128)
            )
            wout = wp.tile([128, KH, OUT], F32R, name="wout_s")
            nc.sync.dma_start(wout[:], wout_d.rearrange("(k p) o -> p k o", p=128))
            bias0 = wp.tile([HC, NM], F32, name="bias0_s")
            nc.sync.dma_start(bias0[:], bias0_d)
            bias1 = wp.tile([HC, NM], F32, name="bias1_s")
            nc.sync.dma_start(bias1[:], bias1_d)
            bout = wp.tile([OUT, 1], F32, name="bout_s")
            nc.sync.dma_start(bout[:], bout_d)

            h0_full = hp.tile([128, KH, B], F32R, name="h0f", tag="h0f")
            nc.sync.dma_start(h0_full[:], h0i_d.rearrange("(k p) n -> p k n", p=128))
            h1_full = hp.tile([128, KH, B], F32R, name="h1f", tag="h1f")
            nc.sync.dma_start(h1_full[:], h1i_d.rearrange("(k p) n -> p k n", p=128))
            c0 = gp.tile([HC, B], F32, name="c0", tag="c0", bufs=2)
            nc.sync.dma_start(c0[:], c0i_d)
            c1 = gp.tile([HC, B], F32, name="c1", tag="c1", bufs=2)
            nc.sync.dma_start(c1[:], c1i_d)
            x = gp.tile([OUT, B], F32R, name="x", tag="x", bufs=2)
            nc.sync.dma_start(x[:], xi_d)

            def mm_hh(g_ps, w_s, h_tile, start):
                # m-outer, k-inner
                for m in range(NM):
                    for k in range(KH):
                        nc.tensor.matmul(
                            g_ps[:, m * B : (m + 1) * B],
                            w_s[:, k, m, :],
                            h_tile[:, k, :],
                            start=(start and k == 0),
                            stop=False,
                        )

            def mm_ih1(g_ps, h_tile):
                # k-outer, m-inner: consume AllGather-DMA'd tiles as they land
                for k in range(KH):
                    for m in range(NM):
                        nc.tensor.matmul(
                            g_ps[:, m * B : (m + 1) * B],
                            wih1[:, k, m, :],
                            h_tile[:, k, :],
                            start=False,
                            stop=(k == KH - 1),
                        )

            def mm_ih0(g_ps, x_tile):
                for m in range(NM):
                    nc.tensor.matmul(
                        g_ps[:, m * B : (m + 1) * B],
                        wih0[:, m, :],
                        x_tile[:],
                        start=False,
                        stop=True,
                    )

            def cell(layer, g_ps, bias_s, c_prev):
                """Cell elementwise math; gate order in g_ps is (i, f, o, g)."""
                ga = gp.tile([HC, GW], F32, name=f"ga{layer}", tag=f"ga{layer}", bufs=1)
                if zero_bias:
                    nc.scalar.activation(
                        ga[:, 0 : 3 * B], g_ps[:, 0 : 3 * B], AF.Sigmoid
                    )
                    nc.scalar.activation(ga[:, 3 * B : GW], g_ps[:, 3 * B : GW], AF.Tanh)
                else:
                    for m, fn in (
                        (0, AF.Sigmoid),
                        (1, AF.Sigmoid),
                        (2, AF.Sigmoid),
                        (3, AF.Tanh),
                    ):
                        nc.scalar.activation(
                            ga[:, m * B : (m + 1) * B],
                            g_ps[:, m * B : (m + 1) * B],
                            fn,
                            bias=bias_s[:, m : m + 1],
                        )
                gi, gf, go, gg = (ga[:, m * B : (m + 1) * B] for m in range(NM))
                t1 = gp.tile([HC, B], F32, name=f"t1_{layer}", tag=f"t1_{layer}", bufs=1)
                nc.vector.tensor_tensor(t1[:], gf, c_prev[:], ALU.mult)
                t2 = gp.tile([HC, B], F32, name=f"t2_{layer}", tag=f"t2_{layer}", bufs=1)
                nc.vector.tensor_tensor(t2[:], gi, gg, ALU.mult)
                c_new = gp.tile([HC, B], F32, name=f"c{layer}", tag=f"c{layer}", bufs=2)
                nc.vector.tensor_tensor(c_new[:], t1[:], t2[:], ALU.add)
                tn = gp.tile([HC, B], F32, name=f"tn{layer}", tag=f"tn{layer}", bufs=1)
                nc.scalar.activation(tn[:], c_new[:], AF.Tanh)
                h_r = gp.tile([HC, B], F32R, name=f"hr{layer}", tag=f"hr{layer}", bufs=2)
                nc.vector.tensor_tensor(h_r[:], go, tn[:], ALU.mult)
                return c_new, h_r

            def allgather(layer, h_r):
                agin = dp.tile([HC, B], F32R, name=f"agin{layer}", tag=f"agin{layer}")
                agout = dp.tile(
                    [H, B],
                    F32R,
                    name=f"agout{layer}",
                    tag=f"agout{layer}",
                    addr_space="Shared",
                )
                nc.sync.dma_start(agin[:], h_r[:])
                nc.gpsimd.collective_compute(
                    "AllGather",
                    ALU.bypass,
                    replica_groups=RG,
                    ins=[agin.opt()],
                    outs=[agout.opt()],
                )
                hf = hp.tile([128, KH, B], F32R, name=f"h{layer}f", tag=f"h{layer}f")
                nc.sync.dma_start(hf[:], agout.rearrange("(k p) n -> p k n", p=128))
                return hf

            def gate_psum():
                return ps.tile([HC, GW], F32, name="gps", tag="gates", bufs=2)

            # ---- prologue: layer 0 of step 0 ----
            g0 = gate_psum()
            mm_hh(g0, whh0, h0_full, start=True)
            mm_ih0(g0, x)
            c0, hr0 = cell(0, g0, bias0, c0)
            h0_full = allgather(0, hr0)

            # ---- main loop (software-pipelined) ----
            for t in range(T):
                g1 = gate_psum()
                mm_hh(g1, whh1, h1_full, start=True)
                mm_ih1(g1, h0_full)
                c1, hr1 = cell(1, g1, bias1, c1)
                h1_full = allgather(1, hr1)

                if t < T - 1:
                    g0 = gate_psum()
                    mm_hh(g0, whh0, h0_full, start=True)

                po = ps.tile([HC, GW], F32, name="po", tag="gates", bufs=2)
                for k in range(KH):
                    nc.tensor.matmul(
                        po[:OUT, :B],
                        wout[:, k, :],
                        h1_full[:, k, :],
                        start=(k == 0),
                        stop=(k == KH - 1),
                    )
                x = gp.tile([OUT, B], F32R, name="x", tag="x", bufs=2)
                nc.scalar.activation(x[:], po[:OUT, :B], AF.Identity, bias=bout[:])
                nc.sync.dma_start(outs_d[t], x[:])

                if t < T - 1:
                    mm_ih0(g0, x)
                    c0, hr0 = cell(0, g0, bias0, c0)
                    h0_full = allgather(0, hr0)

            # ---- state outputs for chunked execution ----
            if n_chunks > 1:
                nc.sync.dma_start(
                    so["h0o"].rearrange("(k p) n -> p k n", p=128), h0_full[:]
                )
                nc.sync.dma_start(
                    so["h1o"].rearrange("(k p) n -> p k n", p=128), h1_full[:]
                )
                nc.sync.dma_start(so["c0o"], c0[:])
                nc.sync.dma_start(so["c1o"], c1[:])
                nc.sync.dma_start(so["xo"], x[:])

    nc.compile()
    return nc


def _get_nc(T, n_chunks, zero_bias):
    key = (T, n_chunks, zero_bias)
    if key not in _CACHE:
        _CACHE[key] = _build(T, n_chunks, zero_bias)
    return _CACHE[key]


def _run(nc, in_maps):
    from concourse import bass_utils

    return bass_utils.run_bass_kernel_spmd(
        nc, in_maps, core_ids=list(range(NCORES))
    ).results


# per-core gate-row order: i, f, o, g (so the sigmoid gates are contiguous)
_GATE_ORDER = (0, 1, 3, 2)


def _rows(r):
    return np.concatenate(
        [np.arange(g * H + r * HC, g * H + (r + 1) * HC) for g in _GATE_ORDER]
    )


def kernel(
    z_style,
    z_skill,
    fc_init_w,
    fc_init_b,
    W_ih0,
    W_hh0,
    b_ih0,
    b_hh0,
    W_ih1,
    W_hh1,
    b_ih1,
    b_hh1,
    fc_out_w,
    fc_out_b,
    T=T_FULL,
    n_chunks=1,
):
    f32 = np.float32
    z = np.concatenate([np.asarray(z_style, f32), np.asarray(z_skill, f32)], axis=1)
    init_flat = z @ np.asarray(fc_init_w, f32).T + np.asarray(fc_init_b, f32)
    h_all = init_flat.reshape(2, -1, H)  # faithful torch-style row-major view
    h0T = np.ascontiguousarray(h_all[0].T)  # [H, B]
    h1T = np.ascontiguousarray(h_all[1].T)

    bias0 = np.asarray(b_ih0, f32) + np.asarray(b_hh0, f32)
    bias1 = np.asarray(b_ih1, f32) + np.asarray(b_hh1, f32)
    zero_bias = not bias0.any() and not bias1.any()
    wout = np.ascontiguousarray(np.asarray(fc_out_w, f32).T)  # [H, OUT]
    bout = np.asarray(fc_out_b, f32).reshape(OUT, 1)

    base_maps = []
    for r in range(NCORES):
        rows = _rows(r)
        base_maps.append(
            {
                "wih0": np.ascontiguousarray(np.asarray(W_ih0, f32)[rows].T),
                "whh0": np.ascontiguousarray(np.asarray(W_hh0, f32)[rows].T),
                "wih1": np.ascontiguousarray(np.asarray(W_ih1, f32)[rows].T),
                "whh1": np.ascontiguousarray(np.asarray(W_hh1, f32)[rows].T),
                "wout": wout,
                "bias0": np.ascontiguousarray(bias0[rows].reshape(NM, HC).T),
                "bias1": np.ascontiguousarray(bias1[rows].reshape(NM, HC).T),
                "bout": bout,
            }
        )

    state = {
        "h0i": h0T,
        "h1i": h1T,
        "xi": np.zeros((OUT, B), f32),
    }
    c0_state = [np.zeros((HC, B), f32)] * NCORES
    c1_state = [np.zeros((HC, B), f32)] * NCORES

    assert T % n_chunks == 0
    t_chunk = T // n_chunks
    nc = _get_nc(t_chunk, n_chunks, zero_bias)

    outs = []
    for _ in range(n_chunks):
        in_maps = []
        for r in range(NCORES):
            m = dict(base_maps[r])
            m.update(
                {
                    "h0i": state["h0i"],
                    "h1i": state["h1i"],
                    "xi": state["xi"],
                    "c0i": c0_state[r],
                    "c1i": c1_state[r],
                }
            )
            in_maps.append(m)
        res = _run(nc, in_maps)
        outs.append(res[0]["outsT"])
        if n_chunks > 1:
            state = {
                "h0i": res[0]["h0o"],
                "h1i": res[0]["h1o"],
                "xi": res[0]["xo"],
            }
            c0_state = [res[r]["c0o"] for r in range(NCORES)]
            c1_state = [res[r]["c1o"] for r in range(NCORES)]
    outsT = np.concatenate(outs, axis=0)  # [T, OUT, B]
    return np.ascontiguousarray(np.transpose(outsT, (2, 0, 1)))  # [B, T, OUT]
